# revision 2
# baseline (speedup 1.0000x reference)
"""GCL (GNN message-passing) Trainium2 Bass kernel on 8 NeuronCores.

Sharding: edges sorted by destination on host and sharded by destination-node
range (1250 nodes/core) -> each core owns the full segment-sum for its nodes,
no collectives. Node features and weights replicated.

Per core, the edge-MLP first-layer split: e1 = A[row] + B[col] where
A = h@we1_top + be1 (per-window SBUF bf16 table, injected via one-hot
matmul) and B = h@we1_bot (bf16 table RESIDENT IN SBUF, 2.56MB; per-edge
rows delivered by SBUF-source dma_gather in transpose mode, which lands
them directly in [D, e] layout -> single identity-matmul inject).

Per core, per 128-destination-node window, edges are processed in 512-edge
macro tiles:
  e1T[D,e] = A @ S_T + I @ BT_gathered               (PSUM accumulate)
  where S_T[n,e] = one-hot(row_local[e]==n) via DVE is_equal (bf16, 4x)
  e2[e,D] = silu(e1) @ we2 + be2                     (be2 via K=1 ones x be2)
  aggT[D,n] += e2^T-scatter via lhsT=e2s, rhs=S      (PSUM accumulate)
Node MLP + residual per 128-node tile, fp32.
"""
import sys
sys.path.insert(0, '/opt/trn_rl_repo')
import numpy as np
import ml_dtypes

N_NODES = 10000
N_EDGES = 640000
D = 128
NORM = 100.0
NCORES = 8
NPC = N_NODES // NCORES          # 1250 destination nodes per core
NWIN = 10                        # 128-node windows per core
CALL = 1024                      # edges per dma_gather call (= 2 macros)
MACRO = 512
PAD_ROWLOCAL = 200.0
NB = 80                          # B table groups: 80*128 = 10240 rows

BF16 = ml_dtypes.bfloat16
_prog_cache = {}


def _wrap_idx16(idx):
    """[n] int -> [128, n/16] int16 wrapped (pos i -> partition i%16, col
    i//16) and replicated into all eight 16-partition groups."""
    n = idx.shape[0]
    block = idx.astype(np.int16).reshape(n // 16, 16).T
    return np.tile(block, (8, 1))


QPAT = [0, 1, 0, 2, 0, 1, 0, 3]   # queue shares ~ 1/cost: cost_q ~ (q+1)


def _build_program(cw_per_window, no_gather=False, no_compute=False,
                   bufs_g=6, bufs_w=3):
    import concourse.bacc as bacc
    import concourse.mybir as mybir
    from concourse import tile

    dt = mybir.dt
    AF = mybir.ActivationFunctionType
    ALU = mybir.AluOpType

    nm_w = [2 * c for c in cw_per_window]
    NM = sum(nm_w)
    NCALLS = sum(cw_per_window)

    nc = bacc.Bacc("TRN2", target_bir_lowering=False, debug=False,
                   num_devices=NCORES, num_swdge_queues=4)

    f32, bf16, i16 = dt.float32, dt.bfloat16, dt.int16
    din = lambda n, s, d=f32: nc.dram_tensor(n, s, d, kind="ExternalInput")
    hT_bf = din("hT_bf", [128, NB * 128], bf16)
    hT_slice = din("hT_slice", [128, NWIN * 128])
    h_slice = din("h_slice", [NWIN, 128, 128])
    we1_top = din("we1_top", [128, 128])
    we1_bot_bf = din("we1_bot_bf", [128, 128], bf16)
    be1_row = din("be1_row", [1, 128])
    we2b_d = din("we2_bf", [128, 128], bf16)
    be2rep4_bf = din("be2rep4_bf", [1, 512], bf16)
    wn1_lo = din("wn1_lo", [128, 128])
    wn1_hi = din("wn1_hi", [128, 128])
    bn1_col = din("bn1_col", [128, 1])
    wn2_d = din("wn2", [128, 128])
    bn2_row = din("bn2_row", [1, 128])
    ones_row = din("ones_row", [1, 128])
    ones_bf_d = din("ones_bf", [1, 128], bf16)
    iota_col_bf_d = din("iota_col_bf", [128, 512], bf16)
    iota_part_d = din("iota_part", [128, 1])
    ident_bf_d = din("ident_bf", [128, 128], bf16)
    ident_f_d = din("ident_f", [128, 128])
    colidx_d = din("colidx", [128, 64 * NCALLS], i16)
    rowloc_c_d = din("rowloc_c", [128, 4 * NM])
    rowloc_r_bf_d = din("rowloc_r_bf", [NM, 512], bf16)
    out_d = nc.dram_tensor("out", [NWIN, 128, 128], f32, kind="ExternalOutput")

    with tile.TileContext(nc) as tc:
        with (
            tc.tile_pool(name="persist", bufs=1) as pp,
            tc.tile_pool(name="work", bufs=bufs_w) as wp,
            tc.tile_pool(name="gout", bufs=bufs_g) as gp,
            tc.tile_pool(name="ps", bufs=2, space="PSUM") as psp,
        ):
            def load(t_dram, shape, dtype=f32):
                t = pp.tile(shape, dtype, tag=t_dram.name)
                nc.sync.dma_start(t[:], t_dram.ap())
                return t

            hTb_t = load(hT_bf, [128, NB * 128], bf16)
            hTs_t = load(hT_slice, [128, NWIN * 128])
            colidx_t = load(colidx_d, [128, 64 * NCALLS], i16)
            rowloc_c = load(rowloc_c_d, [128, 4 * NM])
            w1t = load(we1_top, [128, 128])
            w1b = load(we1_bot_bf, [128, 128], bf16)
            be1r = load(be1_row, [1, 128])
            w2b = load(we2b_d, [128, 128], bf16)
            be2r = load(be2rep4_bf, [1, 512], bf16)
            wn1l = load(wn1_lo, [128, 128])
            wn1h = load(wn1_hi, [128, 128])
            bn1c = load(bn1_col, [128, 1])
            wn2t = load(wn2_d, [128, 128])
            bn2r = load(bn2_row, [1, 128])
            onesr = load(ones_row, [1, 128])
            onesb = load(ones_bf_d, [1, 128], bf16)
            iota_col = load(iota_col_bf_d, [128, 512], bf16)
            iota_part = load(iota_part_d, [128, 1])
            ident_bf = load(ident_bf_d, [128, 128], bf16)
            ident_f = load(ident_f_d, [128, 128])
            hsl_t = pp.tile([128, NWIN, 128], f32, tag="h_slice")
            nc.sync.dma_start(hsl_t[:], h_slice.ap().rearrange("w p d -> p w d"))

            # ---- B table: h @ we1_bot -> bf16, SBUF resident ----
            B_sb = pp.tile([128, NB, 128], bf16, tag="B_sb")
            for q in range(NB // 4):
                bp = psp.tile([128, 512], f32, tag="e1")
                for j in range(4):
                    t = q * 4 + j
                    nc.tensor.matmul(bp[:, j * 128:(j + 1) * 128],
                                     hTb_t[:, t * 128:(t + 1) * 128], w1b[:],
                                     start=True, stop=True,
                                     skip_group_check=True)
                nc.scalar.activation(
                    B_sb[:, q * 4:(q + 1) * 4, :].rearrange("p a b -> p (a b)"),
                    bp[:], AF.Copy)

            # ---- A table: h @ we1_top + be1, single bf16, SBUF resident ----
            a_bf = pp.tile([128, NWIN, 128], bf16, tag="a_bf")
            for w in range(NWIN):
                ap_ = psp.tile([128, 128], f32, tag="e2")
                nc.tensor.matmul(ap_[:], onesr[:], be1r[:], start=True, stop=False)
                nc.tensor.matmul(ap_[:], hTs_t[:, w * 128:(w + 1) * 128], w1t[:],
                                 start=False, stop=True)
                nc.scalar.activation(a_bf[:, w, :], ap_[:], AF.Copy)

            # ---- edge phase ----
            agg_sb = None
            if not no_compute:
                agg_sb = pp.tile([128, NWIN, 128], f32, tag="aggsb")

            # flat macro list: (window, mw-in-window, nmw)
            macros = [(w, mw, nm_w[w]) for w in range(NWIN) for mw in range(nm_w[w])]
            NMtot = len(macros)
            NCH = (NMtot + 3) // 4            # rb chunks of 4 macros
            gts = {}
            rbs = {}
            agg_tiles = {}
            stash = {}
            PREF = 4

            def issue_gather(cc):
                if cc >= NCALLS:
                    return
                gt = gp.tile([128, 1, CALL], bf16, tag="g")
                if not no_gather:
                    nc.gpsimd.dma_gather(
                        gt[:], B_sb[:].rearrange("p a b -> p (a b)"),
                        colidx_t[:, cc * 64:(cc + 1) * 64],
                        num_idxs=CALL, num_idxs_reg=CALL, elem_size=128,
                        transpose=True, single_packet=False,
                        queue_num=QPAT[cc % len(QPAT)],
                        sbuf_tokens_per_rank=128,
                        sbuf_free_dim_per_rank=256,
                        sbuf_free_dim_pad_per_rank=0,
                        sbuf_byte_offset=0,
                    )
                else:
                    nc.vector.tensor_copy(gt[:, 0, 0:8], ident_bf[:, 0:8])
                gts[cc] = gt

            def issue_rb(j):
                if j >= NCH or no_compute:
                    return
                n4 = min(4, NMtot - 4 * j)
                rb = wp.tile([128, 4, 512], bf16, tag="rb")
                src = rowloc_r_bf_d.ap()[4 * j:4 * j + n4, :].rearrange(
                    "(o a) b -> o (a b)", o=1).broadcast_to((128, n4 * 512))
                nc.sync.dma_start(rb[:, 0:n4, :], src)
                rbs[j] = rb

            for p in range(PREF):
                issue_gather(p)
            issue_rb(0)

            def front(i):
                w, mw, nmw = macros[i]
                if i % 2 == 0:
                    issue_gather(i // 2 + PREF)
                if i % 4 == 0 and i > 0:
                    issue_rb(i // 4)
                gt = gts[i // 2]
                if no_compute:
                    if i % 2 == 0:
                        sink = wp.tile([128, 8], bf16, tag="sink")
                        nc.vector.tensor_copy(sink[:], gt[:, 0, 0:8])
                    return
                if i % 4 == 0:
                    issue_rb(i // 4 + 1)
                rb = rbs[i // 4]
                st = wp.tile([128, 512], bf16, tag="st")
                nc.vector.tensor_scalar(
                    st[:], rb[:, i % 4, :], iota_part[:, 0:1], None, ALU.is_equal)
                e1p = psp.tile([128, 512], f32, tag="e1")
                nc.tensor.matmul(e1p[:], a_bf[:, w, :], st[:],
                                 start=True, stop=False, skip_group_check=True)
                nc.tensor.matmul(e1p[:], ident_bf[:],
                                 gt[:, 0, (i % 2) * 512:(i % 2) * 512 + 512],
                                 start=False, stop=True, skip_group_check=True)
                e1s = wp.tile([128, 512], bf16, tag="e1s")
                nc.scalar.activation(e1s[:], e1p[:], AF.Silu)
                stash[i] = e1s

            def back(i):
                if no_compute:
                    return
                w, mw, nmw = macros[i]
                e1s = stash.pop(i)
                if mw == 0:
                    agg_new = psp.tile([128, 128], f32, tag="agg")
                    agg_tiles[w] = agg_new
                agg_ps = agg_tiles[w]
                s4 = wp.tile([128, 512], bf16, tag="s4")
                for t in range(4):
                    nc.vector.tensor_scalar(
                        s4[:, t * 128:(t + 1) * 128],
                        iota_col[:, t * 128:(t + 1) * 128],
                        rowloc_c[:, 4 * i + t:4 * i + t + 1],
                        None, ALU.is_equal)
                e2p = psp.tile([128, 512], f32, tag="e2")
                nc.tensor.matmul(e2p[:], onesb[:], be2r[:],
                                 start=True, stop=False, skip_group_check=True)
                for t in range(4):
                    nc.tensor.matmul(
                        e2p[:, t * 128:(t + 1) * 128],
                        e1s[:, t * 128:(t + 1) * 128], w2b[:],
                        start=False, stop=True, skip_group_check=True)
                e2s = wp.tile([128, 512], bf16, tag="e2s")
                nc.scalar.activation(e2s[:], e2p[:], AF.Silu)
                for t in range(4):
                    nc.tensor.matmul(
                        agg_ps[:],
                        e2s[:, t * 128:(t + 1) * 128],
                        s4[:, t * 128:(t + 1) * 128],
                        start=(mw == 0 and t == 0),
                        stop=(mw == nmw - 1 and t == 3),
                        skip_group_check=True)
                if mw == nmw - 1:
                    nc.scalar.activation(agg_sb[:, w, :], agg_ps[:], AF.Copy,
                                         scale=1.0 / NORM)

            for i in range(NMtot + 1):
                if i < NMtot:
                    front(i)
                if i >= 1:
                    back(i - 1)

            # ---- node phase ----
            if no_compute:
                for w in range(NWIN):
                    nc.sync.dma_start(out_d.ap()[w], hsl_t[:, w, :])
            for w in range(NWIN) if not no_compute else []:
                hp = psp.tile([128, 128], f32, tag="e1")
                nc.tensor.matmul(hp[:], wn1l[:], hTs_t[:, w * 128:(w + 1) * 128],
                                 start=True, stop=False)
                nc.tensor.matmul(hp[:], wn1h[:], agg_sb[:, w, :],
                                 start=False, stop=True)
                hs = wp.tile([128, 128], f32, tag="hs")
                nc.scalar.activation(hs[:], hp[:], AF.Silu, bias=bn1c[:, 0:1])
                op = psp.tile([128, 128], f32, tag="e2")
                nc.tensor.matmul(op[:], onesr[:], bn2r[:], start=True, stop=False)
                nc.tensor.matmul(op[:], hs[:], wn2t[:], start=False, stop=True)
                ot = wp.tile([128, 128], f32, tag="ot")
                nc.vector.tensor_tensor(ot[:], op[:], hsl_t[:, w, :], ALU.add)
                nc.sync.dma_start(out_d.ap()[w], ot[:])

    nc.compile()
    return nc


def _prep_inputs(h, edge_index, we1, be1, we2, be2, wn1, bn1, wn2, bn2):
    """Host-side shard/sort/pad. Returns (cw_per_window, per-core in_maps)."""
    h = np.asarray(h, np.float32)
    row = np.asarray(edge_index[0], np.int64).astype(np.int32)
    col = np.asarray(edge_index[1], np.int64).astype(np.int32)

    # per (core, window) edge lists
    core = row // NPC
    rl_g = row - core * NPC
    win = rl_g // 128
    rl = (rl_g % 128).astype(np.float32)

    counts = np.zeros((NCORES, NWIN), np.int64)
    per = [[None] * NWIN for _ in range(NCORES)]
    for cid in range(NCORES):
        msk = core == cid
        w_c, rl_c, col_c = win[msk], rl[msk], col[msk]
        for w in range(NWIN):
            wm = w_c == w
            per[cid][w] = (col_c[wm], rl_c[wm])
            counts[cid, w] = wm.sum()
    cw_per_window = tuple(int(-(-counts[:, w].max() // CALL)) for w in range(NWIN))

    nm_w = [2 * c for c in cw_per_window]
    NM = sum(nm_w)
    NCALLS = sum(cw_per_window)

    hT_pad = np.zeros((128, NB * 128), np.float32)
    hT_pad[:, :N_NODES] = h.T
    iota_col = np.tile(np.arange(128, dtype=np.float32), 4)[None, :].repeat(128, 0)
    iota_part = np.arange(128, dtype=np.float32)[:, None].copy()
    shared = {
        "hT_bf": hT_pad.astype(BF16),
        "we1_top": np.asarray(we1[:128], np.float32),
        "we1_bot_bf": np.asarray(we1[128:], np.float32).astype(BF16),
        "be1_row": np.asarray(be1, np.float32)[None, :],
        "be2rep4_bf": np.tile(np.asarray(be2, np.float32), 4)[None, :].astype(BF16),
        "wn1_lo": np.asarray(wn1[:128], np.float32),
        "wn1_hi": np.asarray(wn1[128:], np.float32),
        "bn1_col": np.asarray(bn1, np.float32)[:, None].copy(),
        "wn2": np.asarray(wn2, np.float32),
        "bn2_row": np.asarray(bn2, np.float32)[None, :],
        "ones_row": np.ones((1, 128), np.float32),
        "ones_bf": np.ones((1, 128), np.float32).astype(BF16),
        "iota_col_bf": iota_col.astype(BF16),
        "iota_part": iota_part,
        "ident_bf": np.eye(128, dtype=np.float32).astype(BF16),
        "ident_f": np.eye(128, dtype=np.float32),
        "we2_bf": np.asarray(we2, np.float32).astype(BF16),
    }

    in_maps = []
    for cid in range(NCORES):
        colidx = np.zeros((128, 64 * NCALLS), np.int16)
        rowloc_c = np.zeros((128, 4 * NM), np.float32)
        rowloc_r = np.zeros((NM, 512), np.float32)
        ci = 0
        mi = 0
        for w in range(NWIN):
            ccol, crl = per[cid][w]
            n_slots = cw_per_window[w] * CALL
            col_pad = np.zeros(n_slots, np.int32)
            rl_pad = np.full(n_slots, PAD_ROWLOCAL, np.float32)
            col_pad[:len(ccol)] = ccol
            rl_pad[:len(crl)] = crl
            for cc in range(cw_per_window[w]):
                colidx[:, ci * 64:ci * 64 + 64] = _wrap_idx16(
                    col_pad[cc * CALL:(cc + 1) * CALL])
                ci += 1
            for mm in range(2 * cw_per_window[w]):
                seg = rl_pad[mm * MACRO:(mm + 1) * MACRO]
                rowloc_c[:, 4 * mi:4 * mi + 4] = seg.reshape(4, 128).T
                rowloc_r[mi] = seg
                mi += 1
        base = cid * NPC
        hT_slice = hT_pad[:, base:base + NWIN * 128].copy()
        h_slice = np.zeros((NWIN, 128, 128), np.float32)
        hi = min(N_NODES, base + NWIN * 128)
        h_slice.reshape(NWIN * 128, 128)[:hi - base] = h[base:hi]
        in_maps.append({**shared, "hT_slice": hT_slice, "h_slice": h_slice,
                        "colidx": colidx, "rowloc_c": rowloc_c,
                        "rowloc_r_bf": rowloc_r.astype(BF16)})
    return cw_per_window, in_maps


def kernel(**inputs):
    from concourse.bass_utils import run_bass_kernel_spmd

    cw, in_maps = _prep_inputs(**inputs)
    if cw not in _prog_cache:
        _prog_cache[cw] = _build_program(cw)
    nc = _prog_cache[cw]
    res = run_bass_kernel_spmd(nc, in_maps, list(range(NCORES)))
    outs = []
    for cid in range(NCORES):
        o = res.results[cid]["out"].reshape(NWIN * 128, 128)
        outs.append(o[:NPC])
    return np.concatenate(outs, axis=0)[:N_NODES].astype(np.float32)


# revision 6
# speedup vs baseline: 1.3194x; 1.3194x over previous
"""GCL (GNN message-passing) Trainium2 Bass kernel on 8 NeuronCores.

Sharding: edges sorted by destination on host and sharded by destination-node
range (1250 nodes/core) -> each core owns the full segment-sum for its nodes,
no collectives. Node features and weights replicated.

Per core, the edge-MLP first-layer split: e1 = A[row] + B[col] where
A = h@we1_top + be1 (per-window SBUF bf16 table, injected via one-hot
matmul) and B = h@we1_bot (bf16 table RESIDENT IN SBUF, 2.56MB; per-edge
rows delivered by SBUF-source dma_gather in transpose mode, which lands
them directly in [D, e] layout -> single identity-matmul inject).

Per core, per 128-destination-node window, edges are processed in 512-edge
macro tiles:
  e1T[D,e] = A @ S_T + I @ BT_gathered               (PSUM accumulate)
  where S_T[n,e] = one-hot(row_local[e]==n) via DVE is_equal (bf16, 4x)
  e2[e,D] = silu(e1) @ we2 + be2                     (be2 via K=1 ones x be2)
  aggT[D,n] += e2^T-scatter via lhsT=e2s, rhs=S      (PSUM accumulate)
Node MLP + residual per 128-node tile, fp32.
"""
import sys
sys.path.insert(0, '/opt/trn_rl_repo')
import numpy as np
import ml_dtypes

N_NODES = 10000
N_EDGES = 640000
D = 128
NORM = 100.0
NCORES = 8
NPC = N_NODES // NCORES          # 1250 destination nodes per core
NWIN = 10                        # 128-node windows per core
CALL = 1024                      # edges per dma_gather call (= 2 macros)
MACRO = 512
PAD_ROWLOCAL = 200.0
NB = 80                          # B table groups: 80*128 = 10240 rows

BF16 = ml_dtypes.bfloat16
_prog_cache = {}


def _wrap_idx16(idx):
    """[n] int -> [128, n/16] int16 wrapped (pos i -> partition i%16, col
    i//16) and replicated into all eight 16-partition groups."""
    n = idx.shape[0]
    block = idx.astype(np.int16).reshape(n // 16, 16).T
    return np.tile(block, (8, 1))


QPAT = [0, 1, 0, 2, 0, 1, 0, 3]   # queue shares ~ 1/cost: cost_q ~ (q+1)


def _build_program(cw_per_window, no_gather=False, no_compute=False,
                   bufs_g=6, bufs_w=3):
    import concourse.bacc as bacc
    import concourse.mybir as mybir
    from concourse import tile

    dt = mybir.dt
    AF = mybir.ActivationFunctionType
    ALU = mybir.AluOpType

    nm_w = list(cw_per_window)       # macros (512 edges) per window
    NM = sum(nm_w)
    NCALLS = (NM + 1) // 2           # 1024-edge gather calls, flat-packed

    nc = bacc.Bacc("TRN2", target_bir_lowering=False, debug=False,
                   num_devices=NCORES, num_swdge_queues=4)

    f32, bf16, i16 = dt.float32, dt.bfloat16, dt.int16
    din = lambda n, s, d=f32: nc.dram_tensor(n, s, d, kind="ExternalInput")
    hT_bf = din("hT_bf", [128, NB * 128], bf16)
    hT_slice = din("hT_slice", [128, NWIN * 128])
    h_slice = din("h_slice", [NWIN, 128, 128])
    we1_top = din("we1_top", [128, 128])
    we1_bot_bf = din("we1_bot_bf", [128, 128], bf16)
    be1_row = din("be1_row", [1, 128])
    we2b_d = din("we2_bf", [128, 128], bf16)
    be2rep4_bf = din("be2rep4_bf", [1, 512], bf16)
    wn1_lo = din("wn1_lo", [128, 128])
    wn1_hi = din("wn1_hi", [128, 128])
    bn1_col = din("bn1_col", [128, 1])
    wn2_d = din("wn2", [128, 128])
    bn2_row = din("bn2_row", [1, 128])
    ones_row = din("ones_row", [1, 128])
    ones_bf_d = din("ones_bf", [1, 128], bf16)
    iota_col_bf_d = din("iota_col_bf", [128, 512], bf16)
    iota_part_d = din("iota_part", [128, 1])
    ident_bf_d = din("ident_bf", [128, 128], bf16)
    ident_f_d = din("ident_f", [128, 128])
    colidx_d = din("colidx", [128, 64 * NCALLS], i16)
    rowloc_c_d = din("rowloc_c", [128, 4 * NM])
    rowloc_r_bf_d = din("rowloc_r_bf", [NM, 512], bf16)
    out_d = nc.dram_tensor("out", [NWIN, 128, 128], f32, kind="ExternalOutput")

    with tile.TileContext(nc) as tc:
        with (
            tc.tile_pool(name="persist", bufs=1) as pp,
            tc.tile_pool(name="work", bufs=bufs_w) as wp,
            tc.tile_pool(name="gout", bufs=bufs_g) as gp,
            tc.tile_pool(name="ps", bufs=2, space="PSUM") as psp,
        ):
            def load(t_dram, shape, dtype=f32):
                t = pp.tile(shape, dtype, tag=t_dram.name)
                nc.sync.dma_start(t[:], t_dram.ap())
                return t

            hTb_t = load(hT_bf, [128, NB * 128], bf16)
            hTs_t = load(hT_slice, [128, NWIN * 128])
            colidx_t = load(colidx_d, [128, 64 * NCALLS], i16)
            rowloc_c = load(rowloc_c_d, [128, 4 * NM])
            w1t = load(we1_top, [128, 128])
            w1b = load(we1_bot_bf, [128, 128], bf16)
            be1r = load(be1_row, [1, 128])
            w2b = load(we2b_d, [128, 128], bf16)
            be2r = load(be2rep4_bf, [1, 512], bf16)
            wn1l = load(wn1_lo, [128, 128])
            wn1h = load(wn1_hi, [128, 128])
            bn1c = load(bn1_col, [128, 1])
            wn2t = load(wn2_d, [128, 128])
            bn2r = load(bn2_row, [1, 128])
            onesr = load(ones_row, [1, 128])
            onesb = load(ones_bf_d, [1, 128], bf16)
            iota_col = load(iota_col_bf_d, [128, 512], bf16)
            iota_part = load(iota_part_d, [128, 1])
            ident_bf = load(ident_bf_d, [128, 128], bf16)
            ident_f = load(ident_f_d, [128, 128])
            hsl_t = pp.tile([128, NWIN, 128], f32, tag="h_slice")
            nc.sync.dma_start(hsl_t[:], h_slice.ap().rearrange("w p d -> p w d"))

            # ---- B table: h @ we1_bot -> bf16, SBUF resident ----
            B_sb = pp.tile([128, NB, 128], bf16, tag="B_sb")
            for q in range(NB // 4):
                bp = psp.tile([128, 512], f32, tag="e1")
                for j in range(4):
                    t = q * 4 + j
                    nc.tensor.matmul(bp[:, j * 128:(j + 1) * 128],
                                     hTb_t[:, t * 128:(t + 1) * 128], w1b[:],
                                     start=True, stop=True,
                                     skip_group_check=True)
                nc.scalar.activation(
                    B_sb[:, q * 4:(q + 1) * 4, :].rearrange("p a b -> p (a b)"),
                    bp[:], AF.Copy)

            # ---- A table: h @ we1_top + be1, single bf16, SBUF resident ----
            a_bf = pp.tile([128, NWIN, 128], bf16, tag="a_bf")
            for w in range(NWIN):
                ap_ = psp.tile([128, 128], f32, tag="e2")
                nc.tensor.matmul(ap_[:], onesr[:], be1r[:], start=True, stop=False)
                nc.tensor.matmul(ap_[:], hTs_t[:, w * 128:(w + 1) * 128], w1t[:],
                                 start=False, stop=True)
                nc.scalar.activation(a_bf[:, w, :], ap_[:], AF.Copy)

            # ---- edge phase ----
            agg_sb = None
            if not no_compute:
                agg_sb = pp.tile([128, NWIN, 128], f32, tag="aggsb")

            # flat macro list: (window, mw-in-window, nmw)
            macros = [(w, mw, nm_w[w]) for w in range(NWIN) for mw in range(nm_w[w])]
            NMtot = len(macros)
            NCH = (NMtot + 3) // 4            # rb chunks of 4 macros
            gts = {}
            rbs = {}
            agg_tiles = {}
            stash = {}
            PREF = 4

            def issue_gather(cc):
                if cc >= NCALLS:
                    return
                gt = gp.tile([128, 1, CALL], bf16, tag="g")
                if not no_gather:
                    nc.gpsimd.dma_gather(
                        gt[:], B_sb[:].rearrange("p a b -> p (a b)"),
                        colidx_t[:, cc * 64:(cc + 1) * 64],
                        num_idxs=CALL, num_idxs_reg=CALL, elem_size=128,
                        transpose=True, single_packet=False,
                        queue_num=QPAT[cc % len(QPAT)],
                        sbuf_tokens_per_rank=128,
                        sbuf_free_dim_per_rank=256,
                        sbuf_free_dim_pad_per_rank=0,
                        sbuf_byte_offset=0,
                    )
                else:
                    nc.vector.tensor_copy(gt[:, 0, 0:8], ident_bf[:, 0:8])
                gts[cc] = gt

            def issue_rb(j):
                if j >= NCH or no_compute:
                    return
                n4 = min(4, NMtot - 4 * j)
                rb = wp.tile([128, 4, 512], bf16, tag="rb")
                src = rowloc_r_bf_d.ap()[4 * j:4 * j + n4, :].rearrange(
                    "(o a) b -> o (a b)", o=1).broadcast_to((128, n4 * 512))
                nc.sync.dma_start(rb[:, 0:n4, :], src)
                rbs[j] = rb

            for p in range(PREF):
                issue_gather(p)
            issue_rb(0)
            issue_rb(1)

            def front(i):
                w, mw, nmw = macros[i]
                if i % 2 == 0:
                    issue_gather(i // 2 + PREF)
                gt = gts[i // 2]
                if no_compute:
                    if i % 2 == 0:
                        sink = wp.tile([128, 8], bf16, tag="sink")
                        nc.vector.tensor_copy(sink[:], gt[:, 0, 0:8])
                    return
                if i % 4 == 0:
                    issue_rb(i // 4 + 2)
                rb = rbs[i // 4]
                st = wp.tile([128, 512], bf16, tag="st")
                nc.vector.tensor_scalar(
                    st[:], rb[:, i % 4, :], iota_part[:, 0:1], None, ALU.is_equal)
                e1p = psp.tile([128, 512], f32, tag="e1")
                nc.tensor.matmul(e1p[:], a_bf[:, w, :], st[:],
                                 start=True, stop=False, skip_group_check=True)
                nc.tensor.matmul(e1p[:], ident_bf[:],
                                 gt[:, 0, (i % 2) * 512:(i % 2) * 512 + 512],
                                 start=False, stop=True, skip_group_check=True)
                e1s = wp.tile([128, 512], bf16, tag="e1s")
                nc.scalar.activation(e1s[:], e1p[:], AF.Silu)
                stash[i] = e1s

            def back(i):
                if no_compute:
                    return
                w, mw, nmw = macros[i]
                e1s = stash.pop(i)
                if mw == 0:
                    agg_new = psp.tile([128, 128], f32, tag="agg")
                    agg_tiles[w] = agg_new
                agg_ps = agg_tiles[w]
                s4 = wp.tile([128, 512], bf16, tag="s4")
                for t in range(4):
                    nc.vector.tensor_scalar(
                        s4[:, t * 128:(t + 1) * 128],
                        iota_col[:, t * 128:(t + 1) * 128],
                        rowloc_c[:, 4 * i + t:4 * i + t + 1],
                        None, ALU.is_equal)
                e2p = psp.tile([128, 512], f32, tag="e2")
                nc.tensor.matmul(e2p[:], onesb[:], be2r[:],
                                 start=True, stop=False, skip_group_check=True)
                for t in range(4):
                    nc.tensor.matmul(
                        e2p[:, t * 128:(t + 1) * 128],
                        e1s[:, t * 128:(t + 1) * 128], w2b[:],
                        start=False, stop=True, skip_group_check=True)
                e2s = wp.tile([128, 512], bf16, tag="e2s")
                nc.scalar.activation(e2s[:], e2p[:], AF.Silu)
                for t in range(4):
                    nc.tensor.matmul(
                        agg_ps[:],
                        e2s[:, t * 128:(t + 1) * 128],
                        s4[:, t * 128:(t + 1) * 128],
                        start=(mw == 0 and t == 0),
                        stop=(mw == nmw - 1 and t == 3),
                        skip_group_check=True)
                if mw == nmw - 1:
                    nc.scalar.activation(agg_sb[:, w, :], agg_ps[:], AF.Copy,
                                         scale=1.0 / NORM)

            for i in range(NMtot + 1):
                if i < NMtot:
                    front(i)
                if i >= 1:
                    back(i - 1)

            # ---- node phase ----
            if no_compute:
                for w in range(NWIN):
                    nc.sync.dma_start(out_d.ap()[w], hsl_t[:, w, :])
            for w in range(NWIN) if not no_compute else []:
                hp = psp.tile([128, 128], f32, tag="e1")
                nc.tensor.matmul(hp[:], wn1l[:], hTs_t[:, w * 128:(w + 1) * 128],
                                 start=True, stop=False)
                nc.tensor.matmul(hp[:], wn1h[:], agg_sb[:, w, :],
                                 start=False, stop=True)
                hs = wp.tile([128, 128], f32, tag="hs")
                nc.scalar.activation(hs[:], hp[:], AF.Silu, bias=bn1c[:, 0:1])
                op = psp.tile([128, 128], f32, tag="e2")
                nc.tensor.matmul(op[:], onesr[:], bn2r[:], start=True, stop=False)
                nc.tensor.matmul(op[:], hs[:], wn2t[:], start=False, stop=True)
                ot = wp.tile([128, 128], f32, tag="ot")
                nc.vector.tensor_tensor(ot[:], op[:], hsl_t[:, w, :], ALU.add)
                nc.sync.dma_start(out_d.ap()[w], ot[:])

    nc.compile()
    return nc


def _prep_inputs(h, edge_index, we1, be1, we2, be2, wn1, bn1, wn2, bn2):
    """Host-side shard/sort/pad. Returns (cw_per_window, per-core in_maps)."""
    h = np.asarray(h, np.float32)
    row = np.asarray(edge_index[0], np.int64).astype(np.int32)
    col = np.asarray(edge_index[1], np.int64).astype(np.int32)

    # per (core, window) edge lists
    core = row // NPC
    rl_g = row - core * NPC
    win = rl_g // 128
    rl = (rl_g % 128).astype(np.float32)

    counts = np.zeros((NCORES, NWIN), np.int64)
    per = [[None] * NWIN for _ in range(NCORES)]
    for cid in range(NCORES):
        msk = core == cid
        w_c, rl_c, col_c = win[msk], rl[msk], col[msk]
        for w in range(NWIN):
            wm = w_c == w
            per[cid][w] = (col_c[wm], rl_c[wm])
            counts[cid, w] = wm.sum()
    cw_per_window = tuple(int(-(-counts[:, w].max() // MACRO)) for w in range(NWIN))

    nm_w = list(cw_per_window)
    NM = sum(nm_w)
    NCALLS = (NM + 1) // 2

    hT_pad = np.zeros((128, NB * 128), np.float32)
    hT_pad[:, :N_NODES] = h.T
    iota_col = np.tile(np.arange(128, dtype=np.float32), 4)[None, :].repeat(128, 0)
    iota_part = np.arange(128, dtype=np.float32)[:, None].copy()
    shared = {
        "hT_bf": hT_pad.astype(BF16),
        "we1_top": np.asarray(we1[:128], np.float32),
        "we1_bot_bf": np.asarray(we1[128:], np.float32).astype(BF16),
        "be1_row": np.asarray(be1, np.float32)[None, :],
        "be2rep4_bf": np.tile(np.asarray(be2, np.float32), 4)[None, :].astype(BF16),
        "wn1_lo": np.asarray(wn1[:128], np.float32),
        "wn1_hi": np.asarray(wn1[128:], np.float32),
        "bn1_col": np.asarray(bn1, np.float32)[:, None].copy(),
        "wn2": np.asarray(wn2, np.float32),
        "bn2_row": np.asarray(bn2, np.float32)[None, :],
        "ones_row": np.ones((1, 128), np.float32),
        "ones_bf": np.ones((1, 128), np.float32).astype(BF16),
        "iota_col_bf": iota_col.astype(BF16),
        "iota_part": iota_part,
        "ident_bf": np.eye(128, dtype=np.float32).astype(BF16),
        "ident_f": np.eye(128, dtype=np.float32),
        "we2_bf": np.asarray(we2, np.float32).astype(BF16),
    }

    in_maps = []
    for cid in range(NCORES):
        # flat 512-slot macro stream across all windows
        col_all = np.zeros(NCALLS * CALL, np.int32)
        rl_all = np.full(NM * MACRO, PAD_ROWLOCAL, np.float32)
        pos = 0
        for w in range(NWIN):
            ccol, crl = per[cid][w]
            col_all[pos:pos + len(ccol)] = ccol
            rl_all[pos:pos + len(crl)] = crl
            pos += nm_w[w] * MACRO
        colidx = np.zeros((128, 64 * NCALLS), np.int16)
        for cc in range(NCALLS):
            colidx[:, cc * 64:cc * 64 + 64] = _wrap_idx16(
                col_all[cc * CALL:(cc + 1) * CALL])
        rowloc_c = np.zeros((128, 4 * NM), np.float32)
        rowloc_r = rl_all.reshape(NM, MACRO)
        for mi in range(NM):
            rowloc_c[:, 4 * mi:4 * mi + 4] = rowloc_r[mi].reshape(4, 128).T
        base = cid * NPC
        hT_slice = hT_pad[:, base:base + NWIN * 128].copy()
        h_slice = np.zeros((NWIN, 128, 128), np.float32)
        hi = min(N_NODES, base + NWIN * 128)
        h_slice.reshape(NWIN * 128, 128)[:hi - base] = h[base:hi]
        in_maps.append({**shared, "hT_slice": hT_slice, "h_slice": h_slice,
                        "colidx": colidx, "rowloc_c": rowloc_c,
                        "rowloc_r_bf": rowloc_r.astype(BF16)})
    return cw_per_window, in_maps


def kernel(**inputs):
    from concourse.bass_utils import run_bass_kernel_spmd

    cw, in_maps = _prep_inputs(**inputs)
    if cw not in _prog_cache:
        _prog_cache[cw] = _build_program(cw)
    nc = _prog_cache[cw]
    res = run_bass_kernel_spmd(nc, in_maps, list(range(NCORES)))
    outs = []
    for cid in range(NCORES):
        o = res.results[cid]["out"].reshape(NWIN * 128, 128)
        outs.append(o[:NPC])
    return np.concatenate(outs, axis=0)[:N_NODES].astype(np.float32)
